# revision 13
# baseline (speedup 1.0000x reference)
"""DeepseekV2 MLA (weight-absorbed, chunked-softmax MQA) on 8 trn2 NeuronCores.

Sharding: tensor-parallel over heads (16 heads / 8 cores = 2 heads per core);
the 576-wide latent KV cache is replicated per core. Each core computes its two
heads' full attention output [1024, 256]; the host concatenates along the
feature axis. All matmuls run in bf16 with fp32 PSUM accumulation.

KV-side weight absorption (vs the reference's q-side): with T=1024, S=8192,
absorbing w_kc / w_vc into the shared latent cache is far cheaper than the
reference's forward_absorb:
  k_absT[h] = w_kc[h] @ kv_loraT          (PE, [128, S] per head, 1.07 GF)
  v_abs     = kv_lora @ [w_vc[h0]|w_vc[h1]]  ([S, 256] both heads, 2.15 GF)
  scoresT   = k_absT.T @ q_nopeT  (K=128) + kv_ropeT.T @ q_peT  (K=64)
  pT        = exp(scale * scoresT)        (ACT, PSUM->SBUF bf16)
  outT     += v_abs_tile.T @ pT           (PE, accum 64 s-tiles) [128v, 512t]
  denom    += ones.T @ (DVE-accumulated p sums), 1/denomT via K=1 matmuls
  out       = PE-transpose(outT) * 1/denom, DMA out.
Scores contract over 192 dims instead of 576; PV over 128-wide absorbed
values instead of 512-wide latent: ~15 GF/core total vs the baseline's 37.
"""

import os
import sys

import numpy as np
import ml_dtypes

for _p in ("/opt/trn_rl_repo",):
    if os.path.isdir(_p) and _p not in sys.path:
        sys.path.append(_p)

import concourse.bass as bass
import concourse.mybir as mybir
import concourse.tile as tile
from concourse.bass_utils import run_bass_kernel_spmd
from concourse.vector_clock import ScopedClock, VectorClock

# ---------------------------------------------------------------- constants
NOPE, ROPE, LORA, VDIM = 128, 64, 512, 128
T, H, S = 1024, 16, 8192
D = LORA + ROPE            # 576 latent dim
SCALING = (NOPE + ROPE) ** -0.5
N_CORES = 8
HPC = H // N_CORES         # heads per core
NST = S // 128             # 64 s-tiles
NTB = T // 512             # 2 t-blocks
BF16 = mybir.dt.bfloat16
FP32 = mybir.dt.float32
NPBF = ml_dtypes.bfloat16


# ------------------------------------------------- walrus drain workaround
def _patch_tile_drain():
    """The neuronxcc walrus in this container rejects DRAIN instructions
    carrying more than ~2 sync waits ("Too many sync wait commands").
    Split the TileContext exit drain into one drain per processor tick;
    the waits execute sequentially on SP before the all-engine barrier,
    preserving the original semantics."""
    if getattr(tile.TileContext, "_drain_split_patched", False):
        return

    def _drain_and_barrier_split(self, tick_clock, wait_clock):
        gcv = tick_clock.global_clock
        n = len(gcv)
        for proc in range(n):
            t = gcv[proc]
            if t <= 0:
                continue
            vc = VectorClock([0] * n)
            vc.require_at_least(proc, t)
            d = self.nc.sync.drain()
            wait_clock.add_sem_waits(d.ins, ScopedClock({None: vc}))
        self.nc.all_engine_barrier()
        assert self.sems is not None
        popped = self.nc._tile_sem_poison_stack.pop()
        assert popped is self._sem_poison
        self.nc.clear_and_free_semaphores(list(self.sems.allocated().values()))
        self.nc.all_engine_barrier()

    tile.TileContext._drain_and_barrier = _drain_and_barrier_split

    # Same walrus limitation for regular instructions: peel all but the last
    # sync wait off onto same-engine NOPs inserted immediately before the
    # instruction. The engine executes its queue in order, so waiting on the
    # NOPs first is equivalent to one multi-wait instruction.
    orig_add = tile.TileContext._add_instruction

    def _add_instruction_split_waits(self, inst):
        si = inst.sync_info
        if si is not None:
            waits = si.on_wait
            if waits and len(waits) > 1:
                for w in waits[:-1]:
                    nop = mybir.InstNoOp(
                        name=self.nc.get_next_instruction_name(), ins=[], outs=[]
                    )
                    nop.engine = inst.engine
                    nop.sync_info = mybir.SyncInfo(on_wait=[w], on_update=[])
                    orig_add(self, nop)
                inst.sync_info = mybir.SyncInfo(
                    on_wait=[waits[-1]], on_update=si.on_update
                )
        orig_add(self, inst)

    tile.TileContext._add_instruction = _add_instruction_split_waits
    tile.TileContext._drain_split_patched = True


# ------------------------------------------------------------ bass program
MM_KINDS = {}


def _build_program():
    _patch_tile_drain()
    nc = bass.Bass()
    _orig_mm = nc.tensor.matmul

    def _mm_logged(out, lhsT, rhs, kind="?", **kw):
        inst = _orig_mm(out, lhsT, rhs, **kw)
        MM_KINDS[inst.ins.name] = kind
        return inst

    nc.tensor.matmul = _mm_logged
    qnT = nc.declare_dram_parameter("qnT", [HPC, NOPE, T], BF16, isOutput=False)
    qpT = nc.declare_dram_parameter("qpT", [HPC, ROPE, T], BF16, isOutput=False)
    kvT = nc.declare_dram_parameter("kvT", [LORA, S], BF16, isOutput=False)
    kvr = nc.declare_dram_parameter("kvr", [128, S], BF16, isOutput=False)
    wkcT = nc.declare_dram_parameter("wkcT", [HPC, 4, 128, NOPE], BF16, isOutput=False)
    wvc2 = nc.declare_dram_parameter("wvc2", [4, 128, HPC * VDIM], BF16, isOutput=False)
    ident = nc.declare_dram_parameter("ident", [128, 128], BF16, isOutput=False)
    out = nc.declare_dram_parameter("out", [T, HPC * VDIM], FP32, isOutput=True)

    Exp = mybir.ActivationFunctionType.Exp

    with tile.TileContext(nc) as tc:
        with (
            tc.tile_pool(name="res", bufs=1) as res,
            tc.tile_pool(name="ptp", bufs=8) as ptp,
            tc.tile_pool(name="prsum", bufs=4) as prsum,
            tc.tile_pool(name="attnsb", bufs=3) as attnsb,
            tc.tile_pool(name="smsb", bufs=4) as smsb,
            tc.tile_pool(name="outsb", bufs=4) as outsb,
            tc.tile_pool(name="ps_sc", bufs=4, space="PSUM") as ps_sc,
            tc.tile_pool(name="ps_attn", bufs=2, space="PSUM") as ps_attn,
            tc.tile_pool(name="ps_epi", bufs=2, space="PSUM") as ps_epi,
        ):
            # ---------------- resident loads. Block 0's kv chunks are the
            # first item on every DMA queue (they gate the first absorption
            # matmuls after PE warmup); wkcT rides the fast-ramping gpsimd
            # queue ahead of kvr.
            wkcT_sb = res.tile([128, HPC, 4, NOPE], BF16, tag="wkct")
            # kv tiles are allocated PER (lora-chunk, 1024-s block): the tile
            # framework tracks dependencies at tile granularity, so one big
            # [128, S] tile would make the first absorption matmul wait for
            # all eight block DMAs.
            kvT_sb = [
                [
                    res.tile([128, 1024], BF16, tag=f"kvt{c}_{b}",
                             name=f"kvt{c}_{b}")
                    for b in range(8)
                ]
                for c in range(4)
            ]
            kvr_sb = [
                res.tile([128, 1024], BF16, tag=f"kvr{b}", name=f"kvr{b}")
                for b in range(8)
            ]

            # The scalar (ACT) engine issues NO DMAs: its instruction stream
            # must reach the absorption psum-drain copies immediately, or PE
            # stalls on psum rotation waiting for ACT to work through queued
            # DMA-issue instructions.
            def load_kv_block(b):
                sl = slice(b * 1024, (b + 1) * 1024)
                for c in range(4):
                    eng = nc.sync if c < 2 else nc.gpsimd
                    eng.dma_start(
                        kvT_sb[c][b][:], kvT[c * 128:(c + 1) * 128, sl]
                    )
                nc.gpsimd.dma_start(kvr_sb[b][:], kvr[:, sl])

            for h in range(HPC):
                for lc in range(4):
                    nc.gpsimd.dma_start(wkcT_sb[:, h, lc, :], wkcT[h, lc])
            load_kv_block(0)
            wvc2_sb = res.tile([128, 4, HPC * VDIM], BF16, tag="wvc2")
            for lc in range(4):
                nc.gpsimd.dma_start(wvc2_sb[:, lc, :], wvc2[lc])
            ident_sb = res.tile([128, 128], BF16, tag="ident")
            nc.gpsimd.dma_start(ident_sb[:], ident[:, :])
            load_kv_block(1)
            qnT_sb = res.tile([NOPE, HPC * T], BF16, tag="qnt")
            qpT_sb = res.tile([128, HPC * T], BF16, tag="qpt")
            for h in range(HPC):
                nc.sync.dma_start(qnT_sb[:, h * T:(h + 1) * T], qnT[h])
                nc.sync.dma_start(qpT_sb[0:ROPE, h * T:(h + 1) * T], qpT[h])
                nc.sync.dma_start(qpT_sb[ROPE:128, h * T:(h + 1) * T], qpT[h])
            ones_col = res.tile([128, 1], FP32, tag="ones_col")
            nc.vector.memset(ones_col[:], 1.0)
            ones_f32 = res.tile([1, 1], FP32, tag="ones_f32")
            nc.vector.memset(ones_f32[:], 1.0)

            # PE warmup: ~5us of matmuls on local data so HAM un-throttles
            # while kv block 0 streams in
            warm = res.tile([128, 512], BF16, tag="warm")
            nc.vector.memset(warm[:], 0.0)
            wu_ps = ps_sc.tile([128, 512], FP32, tag="sc", name="wu_ps")
            for _w in range(20):
                nc.tensor.matmul(wu_ps[:], warm[:, 0:128], warm[:, 0:512],
                                 kind="warm")

            for b in range(2, 8):
                load_kv_block(b)

            # ---------------- absorption: k_absT[h] = w_kc[h] @ kv_lora.T
            # ([128 nope, S] per head) and v_abs = kv_lora @ [wvc_h0|wvc_h1]
            # ([S, 256], stored s-tile-major). Consumes kv blocks as they land.
            kabsT_sb = [
                res.tile([NOPE, S], BF16, tag=f"kabs{h}", name=f"kabs{h}")
                for h in range(HPC)
            ]
            vabs_sb = res.tile([128, NST, HPC * VDIM], BF16, tag="vabs")

            for b in range(8):
                # k_abs: two 512-wide s-chunks per block per head
                for h in range(HPC):
                    for cc in range(2):
                        c0 = b * 1024 + cc * 512
                        ka_ps = ps_sc.tile([128, 512], FP32, tag="sc",
                                           name="ka_ps")
                        for lc in range(4):
                            nc.tensor.matmul(
                                ka_ps[:],
                                wkcT_sb[:, h, lc, :],
                                kvT_sb[lc][b][:, cc * 512:(cc + 1) * 512],
                                kind="kabs",
                                start=(lc == 0),
                                stop=(lc == 3),
                            )
                        nc.scalar.copy(kabsT_sb[h][:, c0:c0 + 512], ka_ps[:])
                # v_abs: eight 128-wide s-tiles per block, both heads batched
                for st in range(8):
                    s = b * 8 + st
                    va_ps = ps_sc.tile([128, 512], FP32, tag="sc", name="va_ps")
                    for lc in range(4):
                        nc.tensor.matmul(
                            va_ps[:, 0:HPC * VDIM],
                            kvT_sb[lc][b][:, st * 128:(st + 1) * 128],
                            wvc2_sb[:, lc, :],
                            kind="vabs",
                            start=(lc == 0),
                            stop=(lc == 3),
                        )
                    nc.vector.tensor_copy(vabs_sb[:, s, :], va_ps[:, 0:HPC * VDIM])

            # ---------------- main phases: (head, t-block)
            for ph in range(HPC * NTB):
                h, tb = divmod(ph, NTB)
                attn_ps = ps_attn.tile([128, 512], FP32, tag="attn")
                # p-sum accumulators in SBUF: pairs of p tiles are summed in
                # bf16 on DVE (2x fast mode), then folded into two fp32
                # accumulators, one owned by DVE and one by GpSimd, so no
                # single engine serializes the denominator chain
                accA = smsb.tile([128, 512], FP32, tag="acc")
                accB = smsb.tile([128, 512], FP32, tag="acc")
                pending = []  # (ss, pts) of the previous 4-group: PV runs one
                # group behind so its exp dependency is long satisfied
                for sp in range(NST // 4):
                    ss = tuple(4 * sp + k for k in range(4))
                    # rope matmuls for 4 s-tiles back-to-back: K=64 each, in
                    # alternating PE row-groups -> pairs run concurrently and
                    # the row-group exit penalty is paid once per 4 tiles
                    scs = [ps_sc.tile([128, 512], FP32, tag="sc", name="sc_ps")
                           for _ in ss]
                    for i, s in enumerate(ss):
                        lo = (i % 2) * ROPE
                        nc.tensor.matmul(
                            scs[i][:],
                            kvr_sb[s // 8][lo:lo + ROPE,
                                           (s % 8) * 128:(s % 8 + 1) * 128],
                            qpT_sb[lo:lo + ROPE, h * T + tb * 512:h * T + (tb + 1) * 512],
                            kind="rope",
                            start=True,
                            stop=False,
                            tile_position=(lo, 0),
                        )
                    pts = []
                    for i, s in enumerate(ss):
                        nc.tensor.matmul(
                            scs[i][:],
                            kabsT_sb[h][:, s * 128:(s + 1) * 128],
                            qnT_sb[:, h * T + tb * 512:h * T + (tb + 1) * 512],
                            kind="score",
                            start=False,
                            stop=True,
                        )
                        pt = ptp.tile([128, 512], BF16, tag="pt", name="pt")
                        nc.scalar.activation(pt[:], scs[i][:], Exp, scale=SCALING)
                        pts.append(pt)
                    # denominator partials: two bf16 pair-sums (DVE 2x mode),
                    # folded into per-engine accumulators
                    pr01 = prsum.tile([128, 512], BF16, tag="pr")
                    pr23 = prsum.tile([128, 512], BF16, tag="pr")
                    nc.vector.tensor_add(pr01[:], pts[0][:], pts[1][:])
                    nc.vector.tensor_add(pr23[:], pts[2][:], pts[3][:])
                    if sp == 0:
                        nc.vector.tensor_copy(accA[:], pr01[:])
                        nc.gpsimd.tensor_copy(accB[:], pr23[:])
                    else:
                        nc.vector.tensor_add(accA[:], accA[:], pr01[:])
                        nc.gpsimd.tensor_add(accB[:], accB[:], pr23[:])

                    def emit_pv(pv_ss, pv_pts):
                        for i, s in enumerate(pv_ss):
                            nc.tensor.matmul(
                                attn_ps[:],
                                vabs_sb[:, s, h * VDIM:(h + 1) * VDIM],
                                pv_pts[i][:],
                                kind="pv",
                                start=(s == 0),
                                stop=(s == NST - 1),
                            )

                    pending.append((ss, pts))
                    if len(pending) > 1:
                        emit_pv(*pending.pop(0))
                for p in pending:
                    emit_pv(*p)

                # phase epilogue: drain outT, build 1/denomT [128t, 4]
                attn_sb = attnsb.tile([128, 512], BF16, tag="attn")
                nc.scalar.copy(attn_sb[:], attn_ps[:])

                den_ps = ps_epi.tile([1, 512], FP32, tag="epi", name="den_ps")
                nc.tensor.matmul(den_ps[:], ones_col[:], accA[:],
                                 start=True, stop=False)
                nc.tensor.matmul(den_ps[:], ones_col[:], accB[:],
                                 start=False, stop=True)
                den_sb = smsb.tile([1, 512], FP32, tag="den")
                nc.vector.tensor_copy(den_sb[:], den_ps[:])
                dT_ps = ps_epi.tile([128, 4], FP32, tag="epi", name="dT_ps")
                for j in range(4):
                    nc.tensor.matmul(
                        dT_ps[:, j:j + 1],
                        den_sb[0:1, j * 128:(j + 1) * 128],
                        ones_f32[0:1, 0:1],
                    )
                rc = smsb.tile([128, 4], FP32, tag="recip")
                nc.vector.reciprocal(rc[:], dT_ps[:])

                # outT [128v, 512t] -> PE transpose per 128-t chunk -> [t, v],
                # normalize by 1/denom, DMA out
                for j in range(4):
                    tp_ps = ps_epi.tile([128, VDIM], BF16, tag="epi", name="tp_ps")
                    nc.tensor.matmul(
                        tp_ps[:],
                        attn_sb[:, j * 128:(j + 1) * 128],
                        ident_sb[:],
                        kind="trans",
                        is_transpose=True,
                    )
                    ot = outsb.tile([128, VDIM], FP32, tag="out")
                    nc.vector.tensor_scalar_mul(ot[:], tp_ps[:], rc[:, j:j + 1])
                    nc.sync.dma_start(
                        out[tb * 512 + j * 128:tb * 512 + (j + 1) * 128,
                            h * VDIM:(h + 1) * VDIM],
                        ot[:],
                    )
    return nc


_PROGRAM = None


def _get_program():
    global _PROGRAM
    if _PROGRAM is None:
        _PROGRAM = _build_program()
    return _PROGRAM


# ---------------------------------------------------------------- host side
last_results = None  # BassKernelResults of the most recent run (for test.py)


def kernel(q, kv_cache, w_kc, w_vc):
    q = np.asarray(q, dtype=np.float32)
    kv_cache = np.asarray(kv_cache, dtype=np.float32)
    w_kc = np.asarray(w_kc, dtype=np.float32)
    w_vc = np.asarray(w_vc, dtype=np.float32)

    kvT_full = np.ascontiguousarray(kv_cache.T).astype(NPBF)       # [576, S]
    kvT_np = kvT_full[:LORA]                                        # [512, S]
    kvr_np = np.concatenate([kvT_full[LORA:], kvT_full[LORA:]], 0)  # [128, S] rope x2
    ident_np = np.eye(128, dtype=NPBF)

    in_maps = []
    for core in range(N_CORES):
        hs = [core * HPC + i for i in range(HPC)]
        qnT_np = np.stack(
            [np.ascontiguousarray(q[:, h, :NOPE].T) for h in hs]
        ).astype(NPBF)                                              # [HPC,128,T]
        qpT_np = np.stack(
            [np.ascontiguousarray(q[:, h, NOPE:].T) for h in hs]
        ).astype(NPBF)                                              # [HPC,64,T]
        # w_kc[h].T chunked on the lora dim: [HPC, 4, 128 l, 128 nope]
        wkcT_np = np.stack(
            [np.ascontiguousarray(w_kc[h].T).reshape(4, 128, NOPE) for h in hs]
        ).astype(NPBF)
        # w_vc both heads side by side, chunked on lora: [4, 128 l, 256]
        wvc2_np = np.ascontiguousarray(
            np.concatenate(
                [w_vc[h].reshape(4, 128, VDIM) for h in hs], axis=2
            )
        ).astype(NPBF)
        in_maps.append(
            {
                "qnT": qnT_np,
                "qpT": qpT_np,
                "kvT": kvT_np,
                "kvr": kvr_np,
                "wkcT": wkcT_np,
                "wvc2": wvc2_np,
                "ident": ident_np,
            }
        )

    nc = _get_program()
    trace = bool(int(os.environ.get("KERNEL_TRACE", "0")))
    trace_cores = None
    if trace and os.environ.get("KERNEL_TRACE_CORES"):
        trace_cores = [
            int(x) for x in os.environ["KERNEL_TRACE_CORES"].split(",")
        ]
    res = run_bass_kernel_spmd(
        nc,
        in_maps,
        core_ids=list(range(N_CORES)),
        trace=trace,
        trace_cores=trace_cores,
    )
    global last_results
    last_results = res

    full = np.concatenate([res.results[c]["out"] for c in range(N_CORES)], axis=1)
    return np.ascontiguousarray(full.astype(np.float32))


# revision 14
# speedup vs baseline: 198.5450x; 198.5450x over previous
"""DeepseekV2 MLA (weight-absorbed, chunked-softmax MQA) on 8 trn2 NeuronCores.

Sharding: tensor-parallel over heads (16 heads / 8 cores = 2 heads per core);
the 576-wide latent KV cache is replicated per core. Each core computes its two
heads' full attention output [1024, 256]; the host concatenates along the
feature axis. All matmuls run in bf16 with fp32 PSUM accumulation.

KV-side weight absorption (vs the reference's q-side): with T=1024, S=8192,
absorbing w_kc / w_vc into the shared latent cache is far cheaper than the
reference's forward_absorb:
  k_absT[h] = w_kc[h] @ kv_loraT          (PE, [128, S] per head, 1.07 GF)
  v_abs     = kv_lora @ [w_vc[h0]|w_vc[h1]]  ([S, 256] both heads, 2.15 GF)
  scoresT   = k_absT.T @ q_nopeT  (K=128) + kv_ropeT.T @ q_peT  (K=64)
  pT        = exp(scale * scoresT)        (ACT, PSUM->SBUF bf16)
  outT     += v_abs_tile.T @ pT           (PE, accum 64 s-tiles) [128v, 512t]
  denom    += ones.T @ (DVE-accumulated p sums), 1/denomT via K=1 matmuls
  out       = PE-transpose(outT) * 1/denom, DMA out.
Scores contract over 192 dims instead of 576; PV over 128-wide absorbed
values instead of 512-wide latent: ~15 GF/core total vs the baseline's 37.
"""

import os
import sys

import numpy as np
import ml_dtypes

for _p in ("/opt/trn_rl_repo",):
    if os.path.isdir(_p) and _p not in sys.path:
        sys.path.append(_p)

import concourse.bass as bass
import concourse.mybir as mybir
import concourse.tile as tile
from concourse.bass_utils import run_bass_kernel_spmd
from concourse.vector_clock import ScopedClock, VectorClock

# ---------------------------------------------------------------- constants
NOPE, ROPE, LORA, VDIM = 128, 64, 512, 128
T, H, S = 1024, 16, 8192
D = LORA + ROPE            # 576 latent dim
SCALING = (NOPE + ROPE) ** -0.5
N_CORES = 8
HPC = H // N_CORES         # heads per core
NST = S // 128             # 64 s-tiles
NTB = T // 512             # 2 t-blocks
BF16 = mybir.dt.bfloat16
FP32 = mybir.dt.float32
NPBF = ml_dtypes.bfloat16


# ------------------------------------------------- walrus drain workaround
def _patch_tile_drain():
    """The neuronxcc walrus in this container rejects DRAIN instructions
    carrying more than ~2 sync waits ("Too many sync wait commands").
    Split the TileContext exit drain into one drain per processor tick;
    the waits execute sequentially on SP before the all-engine barrier,
    preserving the original semantics."""
    if getattr(tile.TileContext, "_drain_split_patched", False):
        return

    def _drain_and_barrier_split(self, tick_clock, wait_clock):
        gcv = tick_clock.global_clock
        n = len(gcv)
        for proc in range(n):
            t = gcv[proc]
            if t <= 0:
                continue
            vc = VectorClock([0] * n)
            vc.require_at_least(proc, t)
            d = self.nc.sync.drain()
            wait_clock.add_sem_waits(d.ins, ScopedClock({None: vc}))
        self.nc.all_engine_barrier()
        assert self.sems is not None
        popped = self.nc._tile_sem_poison_stack.pop()
        assert popped is self._sem_poison
        self.nc.clear_and_free_semaphores(list(self.sems.allocated().values()))
        self.nc.all_engine_barrier()

    tile.TileContext._drain_and_barrier = _drain_and_barrier_split

    # Same walrus limitation for regular instructions: peel all but the last
    # sync wait off onto same-engine NOPs inserted immediately before the
    # instruction. The engine executes its queue in order, so waiting on the
    # NOPs first is equivalent to one multi-wait instruction.
    orig_add = tile.TileContext._add_instruction

    def _add_instruction_split_waits(self, inst):
        si = inst.sync_info
        if si is not None:
            waits = si.on_wait
            if waits and len(waits) > 1:
                for w in waits[:-1]:
                    nop = mybir.InstNoOp(
                        name=self.nc.get_next_instruction_name(), ins=[], outs=[]
                    )
                    nop.engine = inst.engine
                    nop.sync_info = mybir.SyncInfo(on_wait=[w], on_update=[])
                    orig_add(self, nop)
                inst.sync_info = mybir.SyncInfo(
                    on_wait=[waits[-1]], on_update=si.on_update
                )
        orig_add(self, inst)

    tile.TileContext._add_instruction = _add_instruction_split_waits
    tile.TileContext._drain_split_patched = True


# ------------------------------------------------------------ bass program
MM_KINDS = {}


def _build_program():
    _patch_tile_drain()
    nc = bass.Bass()
    _orig_mm = nc.tensor.matmul

    def _mm_logged(out, lhsT, rhs, kind="?", **kw):
        inst = _orig_mm(out, lhsT, rhs, **kw)
        MM_KINDS[inst.ins.name] = kind
        return inst

    nc.tensor.matmul = _mm_logged
    qnT = nc.declare_dram_parameter("qnT", [HPC, NOPE, T], BF16, isOutput=False)
    qpT = nc.declare_dram_parameter("qpT", [HPC, ROPE, T], BF16, isOutput=False)
    kvT = nc.declare_dram_parameter("kvT", [LORA, S], BF16, isOutput=False)
    kvr = nc.declare_dram_parameter("kvr", [128, S], BF16, isOutput=False)
    wkcT = nc.declare_dram_parameter("wkcT", [HPC, 4, 128, NOPE], BF16, isOutput=False)
    wvc2 = nc.declare_dram_parameter("wvc2", [4, 128, HPC * VDIM], BF16, isOutput=False)
    ident = nc.declare_dram_parameter("ident", [128, 128], BF16, isOutput=False)
    out = nc.declare_dram_parameter("out", [T, HPC * VDIM], FP32, isOutput=True)

    Exp = mybir.ActivationFunctionType.Exp

    with tile.TileContext(nc) as tc:
        with (
            tc.tile_pool(name="res", bufs=1) as res,
            tc.tile_pool(name="ptp", bufs=8) as ptp,
            tc.tile_pool(name="prsum", bufs=4) as prsum,
            tc.tile_pool(name="attnsb", bufs=3) as attnsb,
            tc.tile_pool(name="smsb", bufs=4) as smsb,
            tc.tile_pool(name="outsb", bufs=4) as outsb,
            tc.tile_pool(name="ps_sc", bufs=4, space="PSUM") as ps_sc,
            tc.tile_pool(name="ps_attn", bufs=2, space="PSUM") as ps_attn,
            tc.tile_pool(name="ps_epi", bufs=2, space="PSUM") as ps_epi,
        ):
            # ---------------- resident loads. Block 0's kv chunks are the
            # first item on every DMA queue (they gate the first absorption
            # matmuls after PE warmup); wkcT rides the fast-ramping gpsimd
            # queue ahead of kvr.
            wkcT_sb = res.tile([128, HPC, 4, NOPE], BF16, tag="wkct")
            # kv tiles are allocated PER (lora-chunk, 1024-s block): the tile
            # framework tracks dependencies at tile granularity, so one big
            # [128, S] tile would make the first absorption matmul wait for
            # all eight block DMAs.
            kvT_sb = [
                [
                    res.tile([128, 1024], BF16, tag=f"kvt{c}_{b}",
                             name=f"kvt{c}_{b}")
                    for b in range(8)
                ]
                for c in range(4)
            ]
            kvr_sb = [
                res.tile([128, 1024], BF16, tag=f"kvr{b}", name=f"kvr{b}")
                for b in range(8)
            ]

            # The scalar (ACT) engine issues NO DMAs: its instruction stream
            # must reach the absorption psum-drain copies immediately, or PE
            # stalls on psum rotation waiting for ACT to work through queued
            # DMA-issue instructions.
            def load_kv_block(b):
                sl = slice(b * 1024, (b + 1) * 1024)
                for c in range(4):
                    nc.sync.dma_start(
                        kvT_sb[c][b][:], kvT[c * 128:(c + 1) * 128, sl]
                    )
                nc.gpsimd.dma_start(kvr_sb[b][:], kvr[:, sl])

            for h in range(HPC):
                for lc in range(4):
                    nc.gpsimd.dma_start(wkcT_sb[:, h, lc, :], wkcT[h, lc])
            load_kv_block(0)
            wvc2_sb = res.tile([128, 4, HPC * VDIM], BF16, tag="wvc2")
            for lc in range(4):
                nc.gpsimd.dma_start(wvc2_sb[:, lc, :], wvc2[lc])
            ident_sb = res.tile([128, 128], BF16, tag="ident")
            nc.gpsimd.dma_start(ident_sb[:], ident[:, :])
            load_kv_block(1)
            qnT_sb = res.tile([NOPE, HPC * T], BF16, tag="qnt")
            qpT_sb = res.tile([128, HPC * T], BF16, tag="qpt")
            for h in range(HPC):
                nc.sync.dma_start(qnT_sb[:, h * T:(h + 1) * T], qnT[h])
                nc.sync.dma_start(qpT_sb[0:ROPE, h * T:(h + 1) * T], qpT[h])
                nc.sync.dma_start(qpT_sb[ROPE:128, h * T:(h + 1) * T], qpT[h])
            ones_col = res.tile([128, 1], FP32, tag="ones_col")
            nc.vector.memset(ones_col[:], 1.0)
            ones_f32 = res.tile([1, 1], FP32, tag="ones_f32")
            nc.vector.memset(ones_f32[:], 1.0)

            # PE warmup: ~5us of matmuls on local data so HAM un-throttles
            # while kv block 0 streams in
            warm = res.tile([128, 512], BF16, tag="warm")
            nc.vector.memset(warm[:], 0.0)
            wu_ps = ps_sc.tile([128, 512], FP32, tag="sc", name="wu_ps")
            for _w in range(20):
                nc.tensor.matmul(wu_ps[:], warm[:, 0:128], warm[:, 0:512],
                                 kind="warm")

            for b in range(2, 8):
                load_kv_block(b)

            # ---------------- absorption: k_absT[h] = w_kc[h] @ kv_lora.T
            # ([128 nope, S] per head) and v_abs = kv_lora @ [wvc_h0|wvc_h1]
            # ([S, 256], stored s-tile-major). Consumes kv blocks as they land.
            kabsT_sb = [
                res.tile([NOPE, S], BF16, tag=f"kabs{h}", name=f"kabs{h}")
                for h in range(HPC)
            ]
            vabs_sb = res.tile([128, NST, HPC * VDIM], BF16, tag="vabs")

            for b in range(8):
                # k_abs: two 512-wide s-chunks per block per head
                for h in range(HPC):
                    for cc in range(2):
                        c0 = b * 1024 + cc * 512
                        ka_ps = ps_sc.tile([128, 512], FP32, tag="sc",
                                           name="ka_ps")
                        for lc in range(4):
                            nc.tensor.matmul(
                                ka_ps[:],
                                wkcT_sb[:, h, lc, :],
                                kvT_sb[lc][b][:, cc * 512:(cc + 1) * 512],
                                kind="kabs",
                                start=(lc == 0),
                                stop=(lc == 3),
                            )
                        nc.scalar.copy(kabsT_sb[h][:, c0:c0 + 512], ka_ps[:])
                # v_abs: eight 128-wide s-tiles per block, both heads batched
                for st in range(8):
                    s = b * 8 + st
                    va_ps = ps_sc.tile([128, 512], FP32, tag="sc", name="va_ps")
                    for lc in range(4):
                        nc.tensor.matmul(
                            va_ps[:, 0:HPC * VDIM],
                            kvT_sb[lc][b][:, st * 128:(st + 1) * 128],
                            wvc2_sb[:, lc, :],
                            kind="vabs",
                            start=(lc == 0),
                            stop=(lc == 3),
                        )
                    nc.vector.tensor_copy(vabs_sb[:, s, :], va_ps[:, 0:HPC * VDIM])

            # ---------------- main phases: (head, t-block)
            for ph in range(HPC * NTB):
                h, tb = divmod(ph, NTB)
                attn_ps = ps_attn.tile([128, 512], FP32, tag="attn")
                # p-sum accumulators in SBUF: pairs of p tiles are summed in
                # bf16 on DVE (2x fast mode), then folded into two fp32
                # accumulators, one owned by DVE and one by GpSimd, so no
                # single engine serializes the denominator chain
                accA = smsb.tile([128, 512], FP32, tag="acc")
                accB = smsb.tile([128, 512], FP32, tag="acc")
                pending = []  # (ss, pts) of the previous 4-group: PV runs one
                # group behind so its exp dependency is long satisfied
                for sp in range(NST // 4):
                    ss = tuple(4 * sp + k for k in range(4))
                    # rope matmuls for 4 s-tiles back-to-back: K=64 each, in
                    # alternating PE row-groups -> pairs run concurrently and
                    # the row-group exit penalty is paid once per 4 tiles
                    scs = [ps_sc.tile([128, 512], FP32, tag="sc", name="sc_ps")
                           for _ in ss]
                    for i, s in enumerate(ss):
                        lo = (i % 2) * ROPE
                        nc.tensor.matmul(
                            scs[i][:],
                            kvr_sb[s // 8][lo:lo + ROPE,
                                           (s % 8) * 128:(s % 8 + 1) * 128],
                            qpT_sb[lo:lo + ROPE, h * T + tb * 512:h * T + (tb + 1) * 512],
                            kind="rope",
                            start=True,
                            stop=False,
                            tile_position=(lo, 0),
                        )
                    pts = []
                    for i, s in enumerate(ss):
                        nc.tensor.matmul(
                            scs[i][:],
                            kabsT_sb[h][:, s * 128:(s + 1) * 128],
                            qnT_sb[:, h * T + tb * 512:h * T + (tb + 1) * 512],
                            kind="score",
                            start=False,
                            stop=True,
                        )
                        pt = ptp.tile([128, 512], BF16, tag="pt", name="pt")
                        nc.scalar.activation(pt[:], scs[i][:], Exp, scale=SCALING)
                        pts.append(pt)
                    # denominator partials: two bf16 pair-sums (DVE 2x mode),
                    # folded into per-engine accumulators
                    pr01 = prsum.tile([128, 512], BF16, tag="pr")
                    pr23 = prsum.tile([128, 512], BF16, tag="pr")
                    nc.vector.tensor_add(pr01[:], pts[0][:], pts[1][:])
                    nc.vector.tensor_add(pr23[:], pts[2][:], pts[3][:])
                    if sp == 0:
                        nc.vector.tensor_copy(accA[:], pr01[:])
                        nc.gpsimd.tensor_copy(accB[:], pr23[:])
                    else:
                        nc.vector.tensor_add(accA[:], accA[:], pr01[:])
                        nc.gpsimd.tensor_add(accB[:], accB[:], pr23[:])

                    def emit_pv(pv_ss, pv_pts):
                        for i, s in enumerate(pv_ss):
                            nc.tensor.matmul(
                                attn_ps[:],
                                vabs_sb[:, s, h * VDIM:(h + 1) * VDIM],
                                pv_pts[i][:],
                                kind="pv",
                                start=(s == 0),
                                stop=(s == NST - 1),
                            )

                    pending.append((ss, pts))
                    if len(pending) > 1:
                        emit_pv(*pending.pop(0))
                for p in pending:
                    emit_pv(*p)

                # phase epilogue: drain outT, build 1/denomT [128t, 4]
                attn_sb = attnsb.tile([128, 512], BF16, tag="attn")
                nc.scalar.copy(attn_sb[:], attn_ps[:])

                den_ps = ps_epi.tile([1, 512], FP32, tag="epi", name="den_ps")
                nc.tensor.matmul(den_ps[:], ones_col[:], accA[:],
                                 start=True, stop=False)
                nc.tensor.matmul(den_ps[:], ones_col[:], accB[:],
                                 start=False, stop=True)
                den_sb = smsb.tile([1, 512], FP32, tag="den")
                nc.vector.tensor_copy(den_sb[:], den_ps[:])
                dT_ps = ps_epi.tile([128, 4], FP32, tag="epi", name="dT_ps")
                for j in range(4):
                    nc.tensor.matmul(
                        dT_ps[:, j:j + 1],
                        den_sb[0:1, j * 128:(j + 1) * 128],
                        ones_f32[0:1, 0:1],
                    )
                rc = smsb.tile([128, 4], FP32, tag="recip")
                nc.vector.reciprocal(rc[:], dT_ps[:])

                # outT [128v, 512t] -> PE transpose per 128-t chunk -> [t, v],
                # normalize by 1/denom, DMA out
                for j in range(4):
                    tp_ps = ps_epi.tile([128, VDIM], BF16, tag="epi", name="tp_ps")
                    nc.tensor.matmul(
                        tp_ps[:],
                        attn_sb[:, j * 128:(j + 1) * 128],
                        ident_sb[:],
                        kind="trans",
                        is_transpose=True,
                    )
                    ot = outsb.tile([128, VDIM], FP32, tag="out")
                    nc.vector.tensor_scalar_mul(ot[:], tp_ps[:], rc[:, j:j + 1])
                    nc.sync.dma_start(
                        out[tb * 512 + j * 128:tb * 512 + (j + 1) * 128,
                            h * VDIM:(h + 1) * VDIM],
                        ot[:],
                    )
    return nc


_PROGRAM = None


def _get_program():
    global _PROGRAM
    if _PROGRAM is None:
        _PROGRAM = _build_program()
    return _PROGRAM


# ---------------------------------------------------------------- host side
last_results = None  # BassKernelResults of the most recent run (for test.py)


def kernel(q, kv_cache, w_kc, w_vc):
    q = np.asarray(q, dtype=np.float32)
    kv_cache = np.asarray(kv_cache, dtype=np.float32)
    w_kc = np.asarray(w_kc, dtype=np.float32)
    w_vc = np.asarray(w_vc, dtype=np.float32)

    kvT_full = np.ascontiguousarray(kv_cache.T).astype(NPBF)       # [576, S]
    kvT_np = kvT_full[:LORA]                                        # [512, S]
    kvr_np = np.concatenate([kvT_full[LORA:], kvT_full[LORA:]], 0)  # [128, S] rope x2
    ident_np = np.eye(128, dtype=NPBF)

    in_maps = []
    for core in range(N_CORES):
        hs = [core * HPC + i for i in range(HPC)]
        qnT_np = np.stack(
            [np.ascontiguousarray(q[:, h, :NOPE].T) for h in hs]
        ).astype(NPBF)                                              # [HPC,128,T]
        qpT_np = np.stack(
            [np.ascontiguousarray(q[:, h, NOPE:].T) for h in hs]
        ).astype(NPBF)                                              # [HPC,64,T]
        # w_kc[h].T chunked on the lora dim: [HPC, 4, 128 l, 128 nope]
        wkcT_np = np.stack(
            [np.ascontiguousarray(w_kc[h].T).reshape(4, 128, NOPE) for h in hs]
        ).astype(NPBF)
        # w_vc both heads side by side, chunked on lora: [4, 128 l, 256]
        wvc2_np = np.ascontiguousarray(
            np.concatenate(
                [w_vc[h].reshape(4, 128, VDIM) for h in hs], axis=2
            )
        ).astype(NPBF)
        in_maps.append(
            {
                "qnT": qnT_np,
                "qpT": qpT_np,
                "kvT": kvT_np,
                "kvr": kvr_np,
                "wkcT": wkcT_np,
                "wvc2": wvc2_np,
                "ident": ident_np,
            }
        )

    nc = _get_program()
    trace = bool(int(os.environ.get("KERNEL_TRACE", "0")))
    trace_cores = None
    if trace and os.environ.get("KERNEL_TRACE_CORES"):
        trace_cores = [
            int(x) for x in os.environ["KERNEL_TRACE_CORES"].split(",")
        ]
    res = run_bass_kernel_spmd(
        nc,
        in_maps,
        core_ids=list(range(N_CORES)),
        trace=trace,
        trace_cores=trace_cores,
    )
    global last_results
    last_results = res

    full = np.concatenate([res.results[c]["out"] for c in range(N_CORES)], axis=1)
    return np.ascontiguousarray(full.astype(np.float32))


# revision 16
# speedup vs baseline: 203.0772x; 1.0228x over previous
"""DeepseekV2 MLA (weight-absorbed, chunked-softmax MQA) on 8 trn2 NeuronCores.

Sharding: tensor-parallel over heads (16 heads / 8 cores = 2 heads per core);
the 576-wide latent KV cache is replicated per core. Each core computes its two
heads' full attention output [1024, 256]; the host concatenates along the
feature axis. All matmuls run in bf16 with fp32 PSUM accumulation.

KV-side weight absorption (vs the reference's q-side): with T=1024, S=8192,
absorbing w_kc / w_vc into the shared latent cache is far cheaper than the
reference's forward_absorb:
  k_absT[h] = w_kc[h] @ kv_loraT          (PE, [128, S] per head, 1.07 GF)
  v_abs     = kv_lora @ [w_vc[h0]|w_vc[h1]]  ([S, 256] both heads, 2.15 GF)
  scoresT   = k_absT.T @ q_nopeT  (K=128) + kv_ropeT.T @ q_peT  (K=64)
  pT        = exp(scale * scoresT)        (ACT, PSUM->SBUF bf16)
  outT     += v_abs_tile.T @ pT           (PE, accum 64 s-tiles) [128v, 512t]
  denom    += ones.T @ (DVE-accumulated p sums), 1/denomT via K=1 matmuls
  out       = PE-transpose(outT) * 1/denom, DMA out.
Scores contract over 192 dims instead of 576; PV over 128-wide absorbed
values instead of 512-wide latent: ~15 GF/core total vs the baseline's 37.
"""

import os
import sys

import numpy as np
import ml_dtypes

for _p in ("/opt/trn_rl_repo",):
    if os.path.isdir(_p) and _p not in sys.path:
        sys.path.append(_p)

import concourse.bass as bass
import concourse.mybir as mybir
import concourse.tile as tile
from concourse.bass_utils import run_bass_kernel_spmd
from concourse.vector_clock import ScopedClock, VectorClock

# ---------------------------------------------------------------- constants
NOPE, ROPE, LORA, VDIM = 128, 64, 512, 128
T, H, S = 1024, 16, 8192
D = LORA + ROPE            # 576 latent dim
SCALING = (NOPE + ROPE) ** -0.5
N_CORES = 8
HPC = H // N_CORES         # heads per core
NST = S // 128             # 64 s-tiles
NTB = T // 512             # 2 t-blocks
BF16 = mybir.dt.bfloat16
FP32 = mybir.dt.float32
NPBF = ml_dtypes.bfloat16


# ------------------------------------------------- walrus drain workaround
def _patch_tile_drain():
    """The neuronxcc walrus in this container rejects DRAIN instructions
    carrying more than ~2 sync waits ("Too many sync wait commands").
    Split the TileContext exit drain into one drain per processor tick;
    the waits execute sequentially on SP before the all-engine barrier,
    preserving the original semantics."""
    if getattr(tile.TileContext, "_drain_split_patched", False):
        return

    def _drain_and_barrier_split(self, tick_clock, wait_clock):
        gcv = tick_clock.global_clock
        n = len(gcv)
        for proc in range(n):
            t = gcv[proc]
            if t <= 0:
                continue
            vc = VectorClock([0] * n)
            vc.require_at_least(proc, t)
            d = self.nc.sync.drain()
            wait_clock.add_sem_waits(d.ins, ScopedClock({None: vc}))
        self.nc.all_engine_barrier()
        assert self.sems is not None
        popped = self.nc._tile_sem_poison_stack.pop()
        assert popped is self._sem_poison
        self.nc.clear_and_free_semaphores(list(self.sems.allocated().values()))
        self.nc.all_engine_barrier()

    tile.TileContext._drain_and_barrier = _drain_and_barrier_split

    # Same walrus limitation for regular instructions: peel all but the last
    # sync wait off onto same-engine NOPs inserted immediately before the
    # instruction. The engine executes its queue in order, so waiting on the
    # NOPs first is equivalent to one multi-wait instruction.
    orig_add = tile.TileContext._add_instruction

    def _add_instruction_split_waits(self, inst):
        si = inst.sync_info
        if si is not None:
            waits = si.on_wait
            if waits and len(waits) > 1:
                for w in waits[:-1]:
                    nop = mybir.InstNoOp(
                        name=self.nc.get_next_instruction_name(), ins=[], outs=[]
                    )
                    nop.engine = inst.engine
                    nop.sync_info = mybir.SyncInfo(on_wait=[w], on_update=[])
                    orig_add(self, nop)
                inst.sync_info = mybir.SyncInfo(
                    on_wait=[waits[-1]], on_update=si.on_update
                )
        orig_add(self, inst)

    tile.TileContext._add_instruction = _add_instruction_split_waits
    tile.TileContext._drain_split_patched = True


# ------------------------------------------------------------ bass program
MM_KINDS = {}


def _build_program():
    _patch_tile_drain()
    nc = bass.Bass()
    _orig_mm = nc.tensor.matmul

    def _mm_logged(out, lhsT, rhs, kind="?", **kw):
        inst = _orig_mm(out, lhsT, rhs, **kw)
        MM_KINDS[inst.ins.name] = kind
        return inst

    nc.tensor.matmul = _mm_logged
    qnT = nc.declare_dram_parameter("qnT", [HPC, NOPE, T], BF16, isOutput=False)
    qpT = nc.declare_dram_parameter("qpT", [HPC, ROPE, T], BF16, isOutput=False)
    kvT = nc.declare_dram_parameter("kvT", [LORA, S], BF16, isOutput=False)
    kvr = nc.declare_dram_parameter("kvr", [128, S], BF16, isOutput=False)
    wkcT = nc.declare_dram_parameter("wkcT", [HPC, 4, 128, NOPE], BF16, isOutput=False)
    wvc2 = nc.declare_dram_parameter("wvc2", [4, 128, HPC * VDIM], BF16, isOutput=False)
    ident = nc.declare_dram_parameter("ident", [128, 128], BF16, isOutput=False)
    out = nc.declare_dram_parameter("out", [T, HPC * VDIM], FP32, isOutput=True)

    Exp = mybir.ActivationFunctionType.Exp

    with tile.TileContext(nc) as tc:
        with (
            tc.tile_pool(name="res", bufs=1) as res,
            tc.tile_pool(name="ptp", bufs=8) as ptp,
            tc.tile_pool(name="prsum", bufs=4) as prsum,
            tc.tile_pool(name="attnsb", bufs=3) as attnsb,
            tc.tile_pool(name="smsb", bufs=4) as smsb,
            tc.tile_pool(name="outsb", bufs=4) as outsb,
            tc.tile_pool(name="ps_sc", bufs=4, space="PSUM") as ps_sc,
            tc.tile_pool(name="ps_attn", bufs=2, space="PSUM") as ps_attn,
            tc.tile_pool(name="ps_epi", bufs=2, space="PSUM") as ps_epi,
        ):
            # ---------------- resident loads. Block 0's kv chunks are the
            # first item on every DMA queue (they gate the first absorption
            # matmuls after PE warmup); wkcT rides the fast-ramping gpsimd
            # queue ahead of kvr.
            wkcT_sb = res.tile([128, HPC, 4, NOPE], BF16, tag="wkct")
            # kv tiles are allocated PER (lora-chunk, 1024-s block): the tile
            # framework tracks dependencies at tile granularity, so one big
            # [128, S] tile would make the first absorption matmul wait for
            # all eight block DMAs.
            kvT_sb = [
                [
                    res.tile([128, 1024], BF16, tag=f"kvt{c}_{b}",
                             name=f"kvt{c}_{b}")
                    for b in range(8)
                ]
                for c in range(4)
            ]
            kvr_sb = [
                res.tile([128, 1024], BF16, tag=f"kvr{b}", name=f"kvr{b}")
                for b in range(8)
            ]

            # The scalar (ACT) engine issues NO DMAs: its instruction stream
            # must reach the absorption psum-drain copies immediately, or PE
            # stalls on psum rotation waiting for ACT to work through queued
            # DMA-issue instructions.
            def load_kv_block(b):
                sl = slice(b * 1024, (b + 1) * 1024)
                for c in range(4):
                    nc.sync.dma_start(
                        kvT_sb[c][b][:], kvT[c * 128:(c + 1) * 128, sl]
                    )
                nc.gpsimd.dma_start(kvr_sb[b][:], kvr[:, sl])

            for h in range(HPC):
                for lc in range(4):
                    nc.gpsimd.dma_start(wkcT_sb[:, h, lc, :], wkcT[h, lc])
            load_kv_block(0)
            wvc2_sb = res.tile([128, 4, HPC * VDIM], BF16, tag="wvc2")
            for lc in range(4):
                nc.gpsimd.dma_start(wvc2_sb[:, lc, :], wvc2[lc])
            ident_sb = res.tile([128, 128], BF16, tag="ident")
            nc.gpsimd.dma_start(ident_sb[:], ident[:, :])
            load_kv_block(1)
            qnT_sb = res.tile([NOPE, HPC * T], BF16, tag="qnt")
            qpT_sb = res.tile([128, HPC * T], BF16, tag="qpt")
            for h in range(HPC):
                nc.sync.dma_start(qnT_sb[:, h * T:(h + 1) * T], qnT[h])
                nc.sync.dma_start(qpT_sb[0:ROPE, h * T:(h + 1) * T], qpT[h])
                nc.sync.dma_start(qpT_sb[ROPE:128, h * T:(h + 1) * T], qpT[h])
            ones_col = res.tile([128, 1], BF16, tag="ones_col")
            nc.vector.memset(ones_col[:], 1.0)
            ones_f32 = res.tile([1, 1], FP32, tag="ones_f32")
            nc.vector.memset(ones_f32[:], 1.0)

            # PE warmup: ~5us of matmuls on local data so HAM un-throttles
            # while kv block 0 streams in
            warm = res.tile([128, 512], BF16, tag="warm")
            nc.vector.memset(warm[:], 0.0)
            wu_ps = ps_sc.tile([128, 512], FP32, tag="sc", name="wu_ps")
            for _w in range(12):
                nc.tensor.matmul(wu_ps[:], warm[:, 0:128], warm[:, 0:512],
                                 kind="warm")

            for b in range(2, 8):
                load_kv_block(b)

            # ---------------- absorption: k_absT[h] = w_kc[h] @ kv_lora.T
            # ([128 nope, S] per head) and v_abs = kv_lora @ [wvc_h0|wvc_h1]
            # ([S, 256], stored s-tile-major). Consumes kv blocks as they land.
            kabsT_sb = [
                res.tile([NOPE, S], BF16, tag=f"kabs{h}", name=f"kabs{h}")
                for h in range(HPC)
            ]
            vabs_sb = res.tile([128, NST, HPC * VDIM], BF16, tag="vabs")

            for b in range(8):
                # k_abs: two 512-wide s-chunks per block per head
                for h in range(HPC):
                    for cc in range(2):
                        c0 = b * 1024 + cc * 512
                        ka_ps = ps_sc.tile([128, 512], FP32, tag="sc",
                                           name="ka_ps")
                        for lc in range(4):
                            nc.tensor.matmul(
                                ka_ps[:],
                                wkcT_sb[:, h, lc, :],
                                kvT_sb[lc][b][:, cc * 512:(cc + 1) * 512],
                                kind="kabs",
                                start=(lc == 0),
                                stop=(lc == 3),
                            )
                        nc.scalar.copy(kabsT_sb[h][:, c0:c0 + 512], ka_ps[:])
                # v_abs: eight 128-wide s-tiles per block, both heads batched
                for st in range(8):
                    s = b * 8 + st
                    va_ps = ps_sc.tile([128, 512], FP32, tag="sc", name="va_ps")
                    for lc in range(4):
                        nc.tensor.matmul(
                            va_ps[:, 0:HPC * VDIM],
                            kvT_sb[lc][b][:, st * 128:(st + 1) * 128],
                            wvc2_sb[:, lc, :],
                            kind="vabs",
                            start=(lc == 0),
                            stop=(lc == 3),
                        )
                    nc.vector.tensor_copy(vabs_sb[:, s, :], va_ps[:, 0:HPC * VDIM])

            # ---------------- main phases: (head, t-block)
            for ph in range(HPC * NTB):
                h, tb = divmod(ph, NTB)
                attn_ps = ps_attn.tile([128, 512], FP32, tag="attn")
                # p-sum accumulators in SBUF: pairs of p tiles are summed in
                # bf16 on DVE (2x fast mode), then folded into two fp32
                # accumulators, one owned by DVE and one by GpSimd, so no
                # single engine serializes the denominator chain
                accA = smsb.tile([128, 512], BF16, tag="acc")
                accB = smsb.tile([128, 512], BF16, tag="acc")
                pending = []  # (ss, pts) of the previous 4-group: PV runs one
                # group behind so its exp dependency is long satisfied
                for sp in range(NST // 4):
                    ss = tuple(4 * sp + k for k in range(4))
                    # rope matmuls for 4 s-tiles back-to-back: K=64 each, in
                    # alternating PE row-groups -> pairs run concurrently and
                    # the row-group exit penalty is paid once per 4 tiles
                    scs = [ps_sc.tile([128, 512], FP32, tag="sc", name="sc_ps")
                           for _ in ss]
                    for i, s in enumerate(ss):
                        lo = (i % 2) * ROPE
                        nc.tensor.matmul(
                            scs[i][:],
                            kvr_sb[s // 8][lo:lo + ROPE,
                                           (s % 8) * 128:(s % 8 + 1) * 128],
                            qpT_sb[lo:lo + ROPE, h * T + tb * 512:h * T + (tb + 1) * 512],
                            kind="rope",
                            start=True,
                            stop=False,
                            tile_position=(lo, 0),
                        )
                    pts = []
                    for i, s in enumerate(ss):
                        nc.tensor.matmul(
                            scs[i][:],
                            kabsT_sb[h][:, s * 128:(s + 1) * 128],
                            qnT_sb[:, h * T + tb * 512:h * T + (tb + 1) * 512],
                            kind="score",
                            start=False,
                            stop=True,
                        )
                        pt = ptp.tile([128, 512], BF16, tag="pt", name="pt")
                        nc.scalar.activation(pt[:], scs[i][:], Exp, scale=SCALING)
                        pts.append(pt)
                    # denominator partials: two bf16 pair-sums (DVE 2x mode),
                    # folded into per-engine accumulators
                    pr01 = prsum.tile([128, 512], BF16, tag="pr")
                    pr23 = prsum.tile([128, 512], BF16, tag="pr")
                    nc.vector.tensor_add(pr01[:], pts[0][:], pts[1][:])
                    nc.vector.tensor_add(pr23[:], pts[2][:], pts[3][:])
                    if sp == 0:
                        nc.vector.tensor_copy(accA[:], pr01[:])
                        nc.gpsimd.tensor_copy(accB[:], pr23[:])
                    else:
                        nc.vector.tensor_add(accA[:], accA[:], pr01[:])
                        nc.gpsimd.tensor_add(accB[:], accB[:], pr23[:])

                    def emit_pv(pv_ss, pv_pts):
                        for i, s in enumerate(pv_ss):
                            nc.tensor.matmul(
                                attn_ps[:],
                                vabs_sb[:, s, h * VDIM:(h + 1) * VDIM],
                                pv_pts[i][:],
                                kind="pv",
                                start=(s == 0),
                                stop=(s == NST - 1),
                            )

                    pending.append((ss, pts))
                    if len(pending) > 1:
                        emit_pv(*pending.pop(0))
                for p in pending:
                    emit_pv(*p)

                # phase epilogue: drain outT, build 1/denomT [128t, 4]
                attn_sb = attnsb.tile([128, 512], BF16, tag="attn")
                nc.scalar.copy(attn_sb[:], attn_ps[:])

                den_ps = ps_epi.tile([1, 512], FP32, tag="epi", name="den_ps")
                nc.tensor.matmul(den_ps[:], ones_col[:], accA[:],
                                 start=True, stop=False)
                nc.tensor.matmul(den_ps[:], ones_col[:], accB[:],
                                 start=False, stop=True)
                den_sb = smsb.tile([1, 512], FP32, tag="den")
                nc.vector.tensor_copy(den_sb[:], den_ps[:])
                dT_ps = ps_epi.tile([128, 4], FP32, tag="epi", name="dT_ps")
                for j in range(4):
                    nc.tensor.matmul(
                        dT_ps[:, j:j + 1],
                        den_sb[0:1, j * 128:(j + 1) * 128],
                        ones_f32[0:1, 0:1],
                    )
                rc = smsb.tile([128, 4], FP32, tag="recip")
                nc.vector.reciprocal(rc[:], dT_ps[:])

                # outT [128v, 512t] -> PE transpose per 128-t chunk -> [t, v],
                # normalize by 1/denom, DMA out
                for j in range(4):
                    tp_ps = ps_epi.tile([128, VDIM], BF16, tag="epi", name="tp_ps")
                    nc.tensor.matmul(
                        tp_ps[:],
                        attn_sb[:, j * 128:(j + 1) * 128],
                        ident_sb[:],
                        kind="trans",
                        is_transpose=True,
                    )
                    ot = outsb.tile([128, VDIM], FP32, tag="out")
                    nc.vector.tensor_scalar_mul(ot[:], tp_ps[:], rc[:, j:j + 1])
                    nc.sync.dma_start(
                        out[tb * 512 + j * 128:tb * 512 + (j + 1) * 128,
                            h * VDIM:(h + 1) * VDIM],
                        ot[:],
                    )
    return nc


_PROGRAM = None


def _get_program():
    global _PROGRAM
    if _PROGRAM is None:
        _PROGRAM = _build_program()
    return _PROGRAM


# ---------------------------------------------------------------- host side
last_results = None  # BassKernelResults of the most recent run (for test.py)


def kernel(q, kv_cache, w_kc, w_vc):
    q = np.asarray(q, dtype=np.float32)
    kv_cache = np.asarray(kv_cache, dtype=np.float32)
    w_kc = np.asarray(w_kc, dtype=np.float32)
    w_vc = np.asarray(w_vc, dtype=np.float32)

    kvT_full = np.ascontiguousarray(kv_cache.T).astype(NPBF)       # [576, S]
    kvT_np = kvT_full[:LORA]                                        # [512, S]
    kvr_np = np.concatenate([kvT_full[LORA:], kvT_full[LORA:]], 0)  # [128, S] rope x2
    ident_np = np.eye(128, dtype=NPBF)

    in_maps = []
    for core in range(N_CORES):
        hs = [core * HPC + i for i in range(HPC)]
        qnT_np = np.stack(
            [np.ascontiguousarray(q[:, h, :NOPE].T) for h in hs]
        ).astype(NPBF)                                              # [HPC,128,T]
        qpT_np = np.stack(
            [np.ascontiguousarray(q[:, h, NOPE:].T) for h in hs]
        ).astype(NPBF)                                              # [HPC,64,T]
        # w_kc[h].T chunked on the lora dim: [HPC, 4, 128 l, 128 nope]
        wkcT_np = np.stack(
            [np.ascontiguousarray(w_kc[h].T).reshape(4, 128, NOPE) for h in hs]
        ).astype(NPBF)
        # w_vc both heads side by side, chunked on lora: [4, 128 l, 256]
        wvc2_np = np.ascontiguousarray(
            np.concatenate(
                [w_vc[h].reshape(4, 128, VDIM) for h in hs], axis=2
            )
        ).astype(NPBF)
        in_maps.append(
            {
                "qnT": qnT_np,
                "qpT": qpT_np,
                "kvT": kvT_np,
                "kvr": kvr_np,
                "wkcT": wkcT_np,
                "wvc2": wvc2_np,
                "ident": ident_np,
            }
        )

    nc = _get_program()
    trace = bool(int(os.environ.get("KERNEL_TRACE", "0")))
    trace_cores = None
    if trace and os.environ.get("KERNEL_TRACE_CORES"):
        trace_cores = [
            int(x) for x in os.environ["KERNEL_TRACE_CORES"].split(",")
        ]
    res = run_bass_kernel_spmd(
        nc,
        in_maps,
        core_ids=list(range(N_CORES)),
        trace=trace,
        trace_cores=trace_cores,
    )
    global last_results
    last_results = res

    full = np.concatenate([res.results[c]["out"] for c in range(N_CORES)], axis=1)
    return np.ascontiguousarray(full.astype(np.float32))
